# revision 1
# baseline (speedup 1.0000x reference)
"""Trainium2 Bass kernel for nn_ForceMatchingLoss (batch-data-parallel over 8 NeuronCores).

Full inputs (B=256) are sharded along the batch dimension: core i computes
batches [32*i, 32*i+32) and returns [sum_b fd_b, sum_b cons_b]; the host
sums the 8 partials and divides by 256 (the loss is a batch mean, so the
"all-reduce" is a trivial host-side sum of 8 scalars).
"""

import numpy as np


# ---------------------------------------------------------------------------
# Workaround for this walrus build: CTRL-type instructions (Drain) only accept
# a single sync-wait; TileContext's tail drain aggregates one wait per logical
# processor.  Split the waits across a chain of drains.
# ---------------------------------------------------------------------------
def _install_drain_fix():
    import concourse.tile as tile
    from bass_rust import ScopedClock, SyncInfo

    if getattr(tile.TileContext, "_drain_fix_installed", False):
        return

    def _drain_and_barrier(self, tick_clock, wait_clock):
        drain_inst = self.nc.sync.drain()
        wait_clock.add_sem_waits(
            drain_inst.ins, ScopedClock({None: tick_clock.global_clock})
        )
        si = drain_inst.ins.sync_info
        waits = list(si.on_wait) if si is not None else []
        if len(waits) > 1:
            drain_inst.ins.sync_info = SyncInfo(
                on_wait=waits[:1], on_update=list(si.on_update)
            )
            for i in range(1, len(waits)):
                d = self.nc.sync.drain()
                d.ins.sync_info = SyncInfo(on_wait=waits[i : i + 1], on_update=[])

        self.nc.all_engine_barrier()
        popped = self.nc._tile_sem_poison_stack.pop()
        assert popped is self._sem_poison
        self.nc.clear_and_free_semaphores(list(self.sems.allocated().values()))
        self.nc.all_engine_barrier()

    tile.TileContext._drain_and_barrier = _drain_and_barrier
    tile.TileContext._drain_fix_installed = True


import concourse.bass as bass
import concourse.tile as tile
from concourse import mybir
from concourse.bass import ds, ts
from concourse.masks import make_identity

FP32 = mybir.dt.float32
BF16 = mybir.dt.bfloat16
F32R = mybir.dt.float32r
AX = mybir.AxisListType
ALU = mybir.AluOpType
ACTF = mybir.ActivationFunctionType

B = 32          # batches per core
Q = 16
S = 512
M = 8
D = 128
NCH = 4         # s chunks of 128
GB = 4          # batches per group (packed in one scores psum tile)
NG = B // GB    # 8 groups
SCALE = float(D) ** -0.5
EPS = 1e-8
QD = float(Q * D)


def r(ap):
    return ap.bitcast(F32R)


def build_nc(dtmm="f32r", debug_dump=False, bufs=None):
    nc = bass.Bass("TRN2", target_bir_lowering=False, debug=False)
    q_d = nc.dram_tensor("queries", [B, Q, D], FP32, kind="ExternalInput").ap()
    k_d = nc.dram_tensor("keys", [B, S, D], FP32, kind="ExternalInput").ap()
    v_d = nc.dram_tensor("values", [B, S, D], FP32, kind="ExternalInput").ap()
    kcg_d = nc.dram_tensor("k_cg", [B, M, D], FP32, kind="ExternalInput").ap()
    vcg_d = nc.dram_tensor("v_cg", [B, M, D], FP32, kind="ExternalInput").ap()
    out_d = nc.dram_tensor("out", [1, 2], FP32, kind="ExternalOutput").ap()
    if debug_dump:
        dbg_okb = nc.dram_tensor("dbg_okb", [16, 1024], FP32, kind="ExternalOutput").ap()
        dbg_okcg = nc.dram_tensor("dbg_okcg", [128, 256], FP32, kind="ExternalOutput").ap()
        dbg_acc = nc.dram_tensor("dbg_acc", [128, 128], FP32, kind="ExternalOutput").ap()
        dbg_p = nc.dram_tensor("dbg_p", [128, 512], FP32, kind="ExternalOutput").ap()
        dbg_kt = nc.dram_tensor("dbg_kt", [128, 4, 128], FP32, kind="ExternalOutput").ap()
        dbg_qt = nc.dram_tensor("dbg_qt", [128, 4, 128], FP32, kind="ExternalOutput").ap()
        dbg_sc = nc.dram_tensor("dbg_sc", [128, 512], FP32, kind="ExternalOutput").ap()

    mm = r if dtmm == "f32r" else (lambda ap: ap)
    bf = {"kv": 8, "kt": 5, "sm": 3, "small": 4, "jsb": 4, "okb": 3, "scr": 5,
          "psA": 2, "psJ": 2, "sc_in_psA": 0, "tr_f32r": 0}
    if bufs:
        bf.update(bufs)
    trc = r if bf["tr_f32r"] else (lambda ap: ap)

    with tile.TileContext(nc) as tc:
        with (
            tc.tile_pool(name="const", bufs=1) as constp,
            tc.tile_pool(name="kv", bufs=bf["kv"]) as kvp,
            tc.tile_pool(name="kt", bufs=bf["kt"]) as ktp,
            tc.tile_pool(name="sm", bufs=bf["sm"]) as smp,
            tc.tile_pool(name="small", bufs=bf["small"]) as smallp,
            tc.tile_pool(name="jsb", bufs=bf["jsb"]) as jsbp,
            tc.tile_pool(name="okb", bufs=bf["okb"]) as okbp,
            tc.tile_pool(name="scr", bufs=bf["scr"]) as scrp,
            tc.tile_pool(name="psA", bufs=bf["psA"], space="PSUM") as psA,
            tc.tile_pool(name="psS", bufs=1, space="PSUM") as psS,  # unused if sc_in_psA
            tc.tile_pool(name="psOK", bufs=2, space="PSUM") as psOK,
            tc.tile_pool(name="psJ", bufs=bf["psJ"], space="PSUM") as psJ,
            tc.tile_pool(name="psCG", bufs=1, space="PSUM") as psCG,
        ):
            ident = constp.tile([128, 128], FP32)
            make_identity(nc, ident)
            zeroT = constp.tile([128, 128], FP32)
            nc.scalar.activation(
                out=r(zeroT[:]), in_=ident, func=ACTF.Copy, scale=0.0
            )
            zeroTb = constp.tile([128, 128], BF16)
            nc.scalar.activation(
                out=zeroTb, in_=ident, func=ACTF.Copy, scale=0.0
            )
            # accumulator columns: [dot 0:32 | d2 32:64 | c2 64:96 | cons 96:128]
            accum = constp.tile([128, 128], FP32)
            nc.gpsimd.memset(accum, 0.0)
            ones1 = constp.tile([128, 1], FP32)
            nc.vector.memset(ones1, 1.0)
            ccg_all = constp.tile([8, 32], FP32)

            # ---------- prologue: q transposed, cg tensors ----------
            q_sb = constp.tile([128, 4, 128], FP32)
            nc.sync.dma_start(
                out=q_sb,
                in_=q_d.rearrange("(t b2) q d -> (b2 q) t d", t=4),
            )
            qT = constp.tile([128, 4, 128], BF16)  # [d, t, b2*16+q]
            qtps = psA.tile([128, 512], FP32, tag="ktps")
            for t in range(4):
                nc.tensor.transpose(qtps[:, ts(t, 128)], q_sb[:, t, :], ident)
            # fold the softmax temperature into qT (used by dense + cg scores)
            nc.scalar.activation(
                out=qT[:],
                in_=qtps.rearrange("p (t x) -> p t x", t=4),
                func=ACTF.Copy,
                scale=SCALE,
            )

            kcg_sb = constp.tile([128, 2, 128], FP32)  # [(b2 m), t, d]
            nc.sync.dma_start(
                out=kcg_sb,
                in_=kcg_d.rearrange("(t b2) m d -> (b2 m) t d", t=2),
            )
            kcgT = constp.tile([128, 2, 128], BF16)  # [d, t, b2*8+m]
            kcgtps = psA.tile([128, 512], FP32, tag="ktps")
            for t in range(2):
                nc.tensor.transpose(kcgtps[:, ts(t, 128)], kcg_sb[:, t, :], ident)
            nc.scalar.copy(kcgT[:], kcgtps[:, 0:256].rearrange("p (t x) -> p t x", t=2))

            cgkv2 = constp.tile([8, 32, 256], FP32)  # [m, b, {k|v}]
            nc.sync.dma_start(
                out=r(cgkv2[:, :, 0:128]), in_=r(kcg_d.rearrange("b m d -> m b d"))
            )
            nc.sync.dma_start(
                out=r(cgkv2[:, :, 128:256]), in_=r(vcg_d.rearrange("b m d -> m b d"))
            )

            # ---------- main loop over groups of 4 batches ----------
            for g in range(NG):
                bs = [g * GB + j for j in range(GB)]

                kvs = []
                for b in bs:
                    kv = kvp.tile([128, NCH, 2, 128], FP32, tag="kv")
                    nc.sync.dma_start(
                        out=r(kv[:, :, 0, :]),
                        in_=r(k_d[b].rearrange("(p c) d -> p c d", c=NCH)),
                    )
                    nc.sync.dma_start(
                        out=r(kv[:, :, 1, :]),
                        in_=r(v_d[b].rearrange("(p c) d -> p c d", c=NCH)),
                    )
                    kvs.append(kv)

                # kT per batch via PE transpose
                kts = []
                for j, b in enumerate(bs):
                    ktps = psA.tile([128, 512], FP32, tag="ktps")
                    for c in range(NCH):
                        nc.tensor.transpose(
                            trc(ktps[:, ts(c, 128)]), trc(kvs[j][:, c, 0, :]),
                            trc(ident[:]),
                        )
                    kt = ktp.tile([128, NCH, 128], BF16, tag="kt")
                    if j % 2 == 0:
                        nc.scalar.copy(kt[:], ktps.rearrange("p (c x) -> p c x", c=NCH))
                    else:
                        nc.vector.tensor_copy(kt[:], ktps.rearrange("p (c x) -> p c x", c=NCH))
                    kts.append(kt)

                # scores: 4 batches packed at 32-aligned row offsets
                scps = (psA if bf["sc_in_psA"] else psS).tile(
                    [128, 512], FP32, tag="ktps" if bf["sc_in_psA"] else "scps")
                nc.tensor.matmul(
                    scps,
                    lhsT=zeroTb,
                    rhs=kts[0][:],
                    start=True,
                    stop=False,
                    skip_group_check=True,
                )
                for j, b in enumerate(bs):
                    t, i = b // 8, b % 8
                    nc.tensor.matmul(
                        scps[ds(32 * j, 16), :],
                        lhsT=qT[:, t, ds(16 * i, 16)],
                        rhs=kts[j][:],
                        start=False,
                        stop=True,
                        tile_position=(0, 32 * j),
                        skip_group_check=True,
                    )

                if debug_dump and g == 0:
                    bscr = smp.tile([128, 512], FP32, tag="bounce")
                    nc.vector.tensor_copy(bscr, scps)
                    nc.sync.dma_start(out=dbg_sc, in_=bscr)
                # softmax (no max subtraction: |scores| <= ~7)
                ptil = smp.tile([128, 512], FP32, tag="ptil")
                z = smallp.tile([128, 1], FP32, tag="z")
                nc.scalar.activation(
                    out=ptil, in_=scps, func=ACTF.Exp, accum_out=z
                )
                zr = smallp.tile([128, 1], FP32, tag="zr")
                nc.vector.reciprocal(zr, z)
                nc.vector.tensor_scalar_mul(ptil, ptil, zr)

                # pT via PE transpose
                ptps = psA.tile([128, 512], FP32, tag="ktps")
                for c in range(NCH):
                    nc.tensor.transpose(
                        trc(ptps[:, ts(c, 128)]), trc(ptil[:, ts(c, 128)]),
                        trc(ident[:]),
                    )
                pT = smp.tile([128, NCH, 128], FP32, tag="pT")
                nc.scalar.copy(r(pT[:]), ptps.rearrange("p (c x) -> p c x", c=NCH))

                # c = sum_q p (valid q columns only), scaled by SCALE
                c_t = smallp.tile([128, NCH, GB], FP32, tag="c_t")
                nc.vector.tensor_reduce(
                    out=c_t,
                    in_=pT.rearrange("p c (j w) -> p c j w", j=GB)[:, :, :, 0:16],
                    axis=AX.X,
                    op=ALU.add,
                )
                nc.vector.tensor_scalar_mul(c_t, c_t, SCALE)

                # out/kbar fused: per-batch (16,256) = [kbar | out] halves of
                # two 1-bank (16,512) psum tiles (pair of batches each);
                # base partition 0 so f32r is legal, and the pool
                # double-buffers the pair tiles across groups
                okb = okbp.tile([16, 1024], FP32, tag="okb")  # [-s*kbar | out] x4
                for h in range(2):
                    okps = psOK.tile([16, 512], FP32, tag="okps")
                    for c in range(NCH):
                        for jj in range(2):
                            j = 2 * h + jj
                            nc.tensor.matmul(
                                okps[:, ds(256 * jj, 256)],
                                lhsT=mm(pT[:, c, ds(32 * j, 16)]),
                                rhs=mm(kvs[j][:, c].rearrange("p a x -> p (a x)")),
                                start=(c == 0 and jj == 0),
                                stop=(c == NCH - 1 and jj == 1),
                                skip_group_check=True,
                            )
                    nc.scalar.copy(r(okb[:, ds(512 * h, 512)]), okps)
                for j in range(GB):
                    nc.vector.tensor_scalar_mul(
                        r(okb[:, ds(256 * j, 128)]),
                        okb[:, ds(256 * j, 128)],
                        -SCALE,
                    )

                # ---- coarse-grained chain ----
                cgps = psCG.tile([128, 512], FP32, tag="cgps")
                nc.tensor.matmul(
                    cgps[:, 0:8],
                    lhsT=zeroTb,
                    rhs=kcgT[:, 0, 0:8],
                    start=True,
                    stop=False,
                    skip_group_check=True,
                )
                for j, b in enumerate(bs):
                    t2, i2 = b // 16, b % 16
                    nc.tensor.matmul(
                        cgps[ds(32 * j, 16), 0:8],
                        lhsT=qT[:, b // 8, ds(16 * (b % 8), 16)],
                        rhs=kcgT[:, t2, ds(8 * i2, 8)],
                        start=False,
                        stop=True,
                        tile_position=(0, 32 * j),
                        skip_group_check=True,
                    )
                pcg = smallp.tile([128, 8], FP32, tag="pcg")
                zcg = smallp.tile([128, 1], FP32, tag="zcg")
                nc.scalar.activation(
                    out=pcg, in_=cgps[:, 0:8], func=ACTF.Exp, accum_out=zcg
                )
                zcgr = smallp.tile([128, 1], FP32, tag="zcgr")
                nc.vector.reciprocal(zcgr, zcg)
                nc.vector.tensor_scalar_mul(pcg, pcg, zcgr)
                nc.tensor.transpose(cgps[0:8, ds(8, 128)], pcg, ident)
                pcgT = smallp.tile([8, 128], FP32, tag="pcgT")
                nc.scalar.copy(pcgT[:], cgps[0:8, ds(8, 128)])
                nc.vector.tensor_reduce(
                    out=ccg_all[:, ds(GB * g, GB)],
                    in_=pcgT.rearrange("m (j w) -> m j w", j=GB)[:, :, 0:16],
                    axis=AX.X,
                    op=ALU.add,
                )
                # cg out/kbar: psum cols 256:512 = [kbar_cg | out_cg]
                nc.tensor.matmul(
                    cgps[:, 256:512],
                    lhsT=mm(zeroT[0:8, :]),
                    rhs=mm(cgkv2[:, bs[0], :]),
                    start=True,
                    stop=False,
                    skip_group_check=True,
                )
                for j, b in enumerate(bs):
                    nc.tensor.matmul(
                        cgps[ds(32 * j, 16), 256:512],
                        lhsT=pcgT[:, ds(32 * j, 16)],
                        rhs=cgkv2[:, b, :],
                        start=False,
                        stop=True,
                        tile_position=(0, 32 * j),
                        skip_group_check=True,
                    )
                okcg = okbp.tile([128, 256], FP32, tag="okcg")  # [-s*kbar_cg | out_cg]
                nc.scalar.activation(
                    out=r(okcg[:, 0:128]), in_=cgps[:, 256:384], func=ACTF.Copy,
                    scale=-SCALE,
                )
                nc.scalar.copy(r(okcg[:, 128:256]), cgps[:, 384:512])
                # move each batch's 16 rows down to partitions 0:16 so the
                # consistency sub and jac2_cg see partition-aligned operands
                okcg2 = okbp.tile([16, 1024], FP32, tag="okcg2")
                for j in range(GB):
                    nc.sync.dma_start(
                        out=r(okcg2[:, ds(256 * j, 256)]),
                        in_=r(okcg[ds(32 * j, 16), :]),
                    )

                if debug_dump and g == 0:
                    nc.sync.dma_start(out=dbg_p, in_=ptil)
                    nc.sync.dma_start(out=dbg_okb, in_=okb)
                    nc.sync.dma_start(out=dbg_okcg, in_=okcg)
                # consistency per batch: out_d (okb cols 256j+128) vs out_cg
                for j, b in enumerate(bs):
                    dif = scrp.tile([16, 128], FP32, tag="dif")
                    nc.vector.tensor_sub(
                        dif, okb[:, ds(256 * j + 128, 128)],
                        okcg2[:, ds(256 * j + 128, 128)],
                    )
                    scc = scrp.tile([16, 128], FP32, tag="scc")
                    nc.vector.scalar_tensor_tensor(
                        out=scc, in0=dif, scalar=1.0, in1=dif,
                        op0=ALU.mult, op1=ALU.mult,
                        accum_out=accum[0:16, ds(96 + b, 1)],
                    )

                # ---- per-batch jacobians ----
                for j, b in enumerate(bs):
                    vc = smallp.tile([128, NCH, 128], FP32, tag="vc")
                    nc.gpsimd.tensor_tensor(
                        out=r(vc[:]),
                        in0=kvs[j][:, :, 1, :],
                        in1=c_t[:, :, ds(j, 1)].broadcast_to([128, NCH, 128]),
                        op=ALU.mult,
                    )
                    vccg = smallp.tile([8, 128], FP32, tag="vccg")
                    nc.vector.tensor_scalar(
                        out=r(vccg[:]),
                        in0=cgkv2[:, b, 128:256],
                        scalar1=ccg_all[:, ds(b, 1)],
                        scalar2=SCALE,
                        op0=ALU.mult,
                        op1=ALU.mult,
                    )

                    jp = psJ.tile([128, 512], FP32, tag="jp")
                    for c in range(NCH):
                        nc.tensor.matmul(
                            jp[:, 0:256],
                            lhsT=mm(vc[:, c, :]),
                            rhs=mm(kvs[j][:, c].rearrange("p a x -> p (a x)")),
                            start=(c == 0),
                            stop=False,
                            skip_group_check=True,
                        )
                    nc.tensor.matmul(
                        jp[:, 0:256],
                        lhsT=mm(okb[:, ds(256 * j + 128, 128)]),
                        rhs=mm(okb[:, ds(256 * j, 256)]),
                        start=False,
                        stop=True,
                        skip_group_check=True,
                    )
                    nc.tensor.matmul(
                        jp[:, 256:512],
                        lhsT=mm(vccg),
                        rhs=mm(cgkv2[:, b, :]),
                        start=True,
                        stop=False,
                        skip_group_check=True,
                    )
                    nc.tensor.matmul(
                        jp[:, 256:512],
                        lhsT=mm(okcg2[:, ds(256 * j + 128, 128)]),
                        rhs=mm(okcg2[:, ds(256 * j, 256)]),
                        start=False,
                        stop=True,
                        skip_group_check=True,
                    )

                    jsb = jsbp.tile([128, 2, 128], FP32, tag="jsb")
                    nc.scalar.copy(jsb[:, 0, :], jp[:, 0:128])
                    nc.scalar.copy(jsb[:, 1, :], jp[:, 256:384])

                    s1 = scrp.tile([128, 128], FP32, tag="s1")
                    nc.vector.scalar_tensor_tensor(
                        out=s1, in0=jsb[:, 0, :], scalar=1.0, in1=jsb[:, 1, :],
                        op0=ALU.mult, op1=ALU.mult,
                        accum_out=accum[:, ds(b, 1)],
                    )
                    s2 = scrp.tile([128, 128], FP32, tag="s2")
                    nc.vector.scalar_tensor_tensor(
                        out=s2, in0=jsb[:, 1, :], scalar=1.0, in1=jsb[:, 1, :],
                        op0=ALU.mult, op1=ALU.mult,
                        accum_out=accum[:, ds(64 + b, 1)],
                    )
                    s3 = scrp.tile([128, 128], FP32, tag="s3")
                    nc.vector.scalar_tensor_tensor(
                        out=s3, in0=jsb[:, 0, :], scalar=1.0, in1=jsb[:, 0, :],
                        op0=ALU.mult, op1=ALU.mult,
                        accum_out=accum[:, ds(32 + b, 1)],
                    )

            # ---------- final reduction ----------
            if debug_dump:
                nc.sync.dma_start(out=dbg_acc, in_=accum)
            # partition reduction via ones-vector matmul (gpsimd C-reduce is
            # pathologically slow on hardware)
            rps = psJ.tile([1, 128], FP32, tag="jp")
            nc.tensor.matmul(
                rps, lhsT=ones1, rhs=accum, start=True, stop=True,
                skip_group_check=True,
            )
            row = constp.tile([1, 128], FP32)
            nc.scalar.copy(row, rps)
            f1 = constp.tile([1, 32], FP32)
            nc.vector.tensor_tensor(
                out=f1, in0=row[:, 32:64], in1=row[:, 64:96], op=ALU.mult
            )
            nc.scalar.activation(out=f1, in_=f1, func=ACTF.Sqrt)
            nc.vector.tensor_scalar_add(f1, f1, EPS)
            f2 = constp.tile([1, 32], FP32)
            nc.vector.reciprocal(f2, f1)
            nc.vector.tensor_tensor(
                out=f2, in0=row[:, 0:32], in1=f2, op=ALU.mult
            )
            csum = constp.tile([1, 1], FP32)
            nc.vector.tensor_reduce(out=csum, in_=f2, axis=AX.X, op=ALU.add)
            msum = constp.tile([1, 1], FP32)
            nc.vector.tensor_reduce(
                out=msum, in_=row[:, 96:128], axis=AX.X, op=ALU.add
            )
            part = constp.tile([1, 2], FP32)
            nc.vector.tensor_scalar(
                out=part[:, 0:1], in0=csum, scalar1=-1.0, scalar2=float(B),
                op0=ALU.mult, op1=ALU.add,
            )
            nc.vector.tensor_scalar_mul(part[:, 1:2], msum, 1.0 / QD)
            nc.sync.dma_start(out=out_d, in_=part)

    return nc



_NC_CACHE = {}


def _get_nc():
    if "nc" not in _NC_CACHE:
        _install_drain_fix()
        nc = build_nc()
        _split_waits(nc)
        _NC_CACHE["nc"] = nc
    return _NC_CACHE["nc"]


def _split_waits(nc):
    """This walrus accepts only one sync-wait per instruction; move extras
    onto same-engine NoOps inserted just before."""
    from concourse import mybir
    from bass_rust import SyncInfo

    for f in nc.m.functions:
        for blk in f.blocks:
            insts = list(blk.instructions)
            out = []
            for inst in insts:
                si = inst.sync_info
                waits = list(si.on_wait) if si is not None else []
                if len(waits) > 1:
                    for wi, w in enumerate(waits[:-1]):
                        nop = mybir.InstNoOp(name=f"{inst.name}-wsplit{wi}")
                        nop.engine = inst.engine
                        nop.sync_info = SyncInfo(on_wait=[w], on_update=[])
                        out.append(nop)
                    inst.sync_info = SyncInfo(
                        on_wait=[waits[-1]], on_update=list(si.on_update)
                    )
                out.append(inst)
            blk.instructions = out


N_CORES = 8


def kernel(queries, keys, values, k_cg, v_cg):
    from concourse.bass_utils import run_bass_kernel_spmd

    queries = np.ascontiguousarray(np.asarray(queries, dtype=np.float32))
    keys = np.ascontiguousarray(np.asarray(keys, dtype=np.float32))
    values = np.ascontiguousarray(np.asarray(values, dtype=np.float32))
    k_cg = np.ascontiguousarray(np.asarray(k_cg, dtype=np.float32))
    v_cg = np.ascontiguousarray(np.asarray(v_cg, dtype=np.float32))

    nb = queries.shape[0]
    sh = nb // N_CORES
    in_maps = [
        {
            "queries": queries[i * sh : (i + 1) * sh],
            "keys": keys[i * sh : (i + 1) * sh],
            "values": values[i * sh : (i + 1) * sh],
            "k_cg": k_cg[i * sh : (i + 1) * sh],
            "v_cg": v_cg[i * sh : (i + 1) * sh],
        }
        for i in range(N_CORES)
    ]
    nc = _get_nc()
    res = run_bass_kernel_spmd(nc, in_maps, core_ids=list(range(N_CORES)))
    total = 0.0
    for i in range(N_CORES):
        part = res.results[i]["out"]
        total += float(part[0, 0]) + float(part[0, 1])
    return np.float32(total / nb)



# revision 11
# speedup vs baseline: 1.3874x; 1.3874x over previous
"""Trainium2 Bass kernel for nn_ForceMatchingLoss (batch-data-parallel over 8 NeuronCores).

Phase B on top of kernel_a:
- software-pipelined kT transposes (group g+1's transposes fill the PE gap
  during group g's softmax latency chain)
- one kv DMA per tensor per group (4 batches each, 2KB descriptors)
- bf16 k copies feed bf16 transposes (1 cyc/col) and a 128-wide bf16 jac1
  (dropping the wasted v*v^T half of the f32r pairing trick)
- kbar scaling folded into the strided psum->sbuf copies
- jac reductions: one staging copy + one DVE dot + two scalar Square accums
"""

import numpy as np


def _install_drain_fix():
    import concourse.tile as tile
    from bass_rust import ScopedClock, SyncInfo

    if getattr(tile.TileContext, "_drain_fix_installed", False):
        return

    def _drain_and_barrier(self, tick_clock, wait_clock):
        drain_inst = self.nc.sync.drain()
        wait_clock.add_sem_waits(
            drain_inst.ins, ScopedClock({None: tick_clock.global_clock})
        )
        si = drain_inst.ins.sync_info
        waits = list(si.on_wait) if si is not None else []
        if len(waits) > 1:
            drain_inst.ins.sync_info = SyncInfo(
                on_wait=waits[:1], on_update=list(si.on_update)
            )
            for i in range(1, len(waits)):
                d = self.nc.sync.drain()
                d.ins.sync_info = SyncInfo(on_wait=waits[i : i + 1], on_update=[])

        self.nc.all_engine_barrier()
        popped = self.nc._tile_sem_poison_stack.pop()
        assert popped is self._sem_poison
        self.nc.clear_and_free_semaphores(list(self.sems.allocated().values()))
        self.nc.all_engine_barrier()

    tile.TileContext._drain_and_barrier = _drain_and_barrier
    tile.TileContext._drain_fix_installed = True


import concourse.bass as bass
import concourse.tile as tile
from concourse import mybir
from concourse.bass import ds, ts
from concourse.masks import make_identity

FP32 = mybir.dt.float32
BF16 = mybir.dt.bfloat16
F32R = mybir.dt.float32r
AX = mybir.AxisListType
ALU = mybir.AluOpType
ACTF = mybir.ActivationFunctionType

B = 32
Q = 16
S = 512
M = 8
D = 128
NCH = 4
GB = 4
NG = B // GB
SCALE = float(D) ** -0.5
EPS = 1e-8
QD = float(Q * D)


def r(ap):
    return ap.bitcast(F32R)


def build_nc(debug_dump=False, bufs=None):
    nc = bass.Bass("TRN2", target_bir_lowering=False, debug=False)
    q_d = nc.dram_tensor("queries", [B, Q, D], FP32, kind="ExternalInput").ap()
    k_d = nc.dram_tensor("keys", [B, S, D], FP32, kind="ExternalInput").ap()
    v_d = nc.dram_tensor("values", [B, S, D], FP32, kind="ExternalInput").ap()
    kcg_d = nc.dram_tensor("k_cg", [B, M, D], FP32, kind="ExternalInput").ap()
    vcg_d = nc.dram_tensor("v_cg", [B, M, D], FP32, kind="ExternalInput").ap()
    out_d = nc.dram_tensor("out", [1, 2], FP32, kind="ExternalOutput").ap()

    mm = r
    bf = {"kv": 3, "kbf": 3, "kt": 6, "sm": 3, "small": 4, "okb": 3, "scr": 4,
          "psA": 2, "psJ": 2}
    if bufs:
        bf.update(bufs)

    with tile.TileContext(nc) as tc:
        with (
            tc.tile_pool(name="const", bufs=1) as constp,
            tc.tile_pool(name="kv", bufs=bf["kv"]) as kvp,
            tc.tile_pool(name="kbf", bufs=bf["kbf"]) as kbfp,
            tc.tile_pool(name="kt", bufs=bf["kt"]) as ktp,
            tc.tile_pool(name="sm", bufs=bf["sm"]) as smp,
            tc.tile_pool(name="small", bufs=bf["small"]) as smallp,
            tc.tile_pool(name="okb", bufs=bf["okb"]) as okbp,
            tc.tile_pool(name="scr", bufs=bf["scr"]) as scrp,
            tc.tile_pool(name="psA", bufs=bf["psA"], space="PSUM") as psA,
            tc.tile_pool(name="psS", bufs=1, space="PSUM") as psS,
            tc.tile_pool(name="psOK", bufs=2, space="PSUM") as psOK,
            tc.tile_pool(name="psJ", bufs=bf["psJ"], space="PSUM") as psJ,
            tc.tile_pool(name="psCG", bufs=1, space="PSUM") as psCG,
        ):
            ident = constp.tile([128, 128], FP32)
            make_identity(nc, ident)
            identb = constp.tile([128, 128], BF16)
            nc.scalar.copy(identb, ident)
            identr = constp.tile([128, 128], FP32)
            nc.scalar.activation(
                out=r(identr[:]), in_=ident, func=ACTF.Copy, scale=1.0
            )
            # accumulator columns: [dot 0:32 | d2 32:64 | c2 64:96 | cons 96:104]
            accum = constp.tile([128, 128], FP32)
            nc.gpsimd.memset(accum, 0.0)
            ones1 = constp.tile([128, 1], FP32)
            nc.vector.memset(ones1, 1.0)
            ccg_all = constp.tile([8, 32], FP32)

            # ---------- prologue ----------
            q_sb = constp.tile([128, 4, 128], FP32)
            nc.sync.dma_start(
                out=r(q_sb[:]),
                in_=r(q_d.rearrange("(t b2) q d -> (b2 q) t d", t=4)),
            )
            qT = constp.tile([128, 4, 128], BF16)  # [d, t, b2*16+q]
            qtps = psS.tile([128, 512], FP32, tag="scps")
            for t in range(4):
                nc.tensor.transpose(
                    r(qtps[:, ts(t, 128)]), r(q_sb[:, t, :]), r(identr[:])
                )
            nc.scalar.activation(
                out=qT[:],
                in_=qtps.rearrange("p (t x) -> p t x", t=4),
                func=ACTF.Copy,
                scale=SCALE,
            )

            kcg_sb = constp.tile([128, 2, 128], FP32)  # [(b2 m), t, d]
            nc.sync.dma_start(
                out=r(kcg_sb[:]),
                in_=r(kcg_d.rearrange("(t b2) m d -> (b2 m) t d", t=2)),
            )
            kcgT = constp.tile([128, 2, 128], BF16)  # [d, t, b2*8+m]
            kcgtps = psS.tile([128, 512], FP32, tag="scps")
            for t in range(2):
                nc.tensor.transpose(
                    r(kcgtps[:, ts(t, 128)]), r(kcg_sb[:, t, :]), r(identr[:])
                )
            nc.scalar.copy(kcgT[:], kcgtps[:, 0:256].rearrange("p (t x) -> p t x", t=2))

            cgkv2 = constp.tile([8, 32, 256], FP32)  # [m, b, {k|v}]
            nc.sync.dma_start(
                out=cgkv2[:, :, 0:128], in_=kcg_d.rearrange("b m d -> m b d")
            )
            nc.sync.dma_start(
                out=cgkv2[:, :, 128:256], in_=vcg_d.rearrange("b m d -> m b d")
            )
            cgkv2b = constp.tile([8, 32, 256], BF16)
            nc.scalar.copy(cgkv2b[:, 0:8], cgkv2[:, 0:8])
            nc.vector.tensor_copy(cgkv2b[:, 8:16], cgkv2[:, 8:16])

            # ---------- pipelined helpers ----------
            def emit_loads(g):
                b0 = g * GB
                kvg = kvp.tile([128, GB, 2, NCH, 128], FP32, tag="kv")
                nc.sync.dma_start(
                    out=r(kvg[:, :, 0].rearrange("p b c d -> p b (c d)")),
                    in_=r(k_d[b0:b0 + GB].rearrange("b (p c) d -> p b (c d)", c=NCH)),
                )
                nc.sync.dma_start(
                    out=r(kvg[:, :, 1].rearrange("p b c d -> p b (c d)")),
                    in_=r(v_d[b0:b0 + GB].rearrange("b (p c) d -> p b (c d)", c=NCH)),
                )
                return kvg

            def emit_cast(kvg):
                kbf = kbfp.tile([128, GB, NCH, 128], BF16, tag="kbf")
                nc.scalar.copy(kbf[:, 0:2], kvg[:, 0:2, 0])
                nc.vector.tensor_copy(kbf[:, 2:4], kvg[:, 2:4, 0])
                return kbf

            def emit_kt(kbf):
                kts = []
                for j in range(GB):
                    ktps = psA.tile([128, 1024], BF16, tag="ktpsb")
                    for c in range(NCH):
                        nc.tensor.transpose(
                            ktps[:, ts(c, 128)], kbf[:, j, c, :], identb
                        )
                    kt = ktp.tile([128, NCH, 128], BF16, tag="kt")
                    if j % 2 == 0:
                        nc.scalar.copy(kt[:], ktps[:, 0:512].rearrange("p (c x) -> p c x", c=NCH))
                    else:
                        nc.vector.tensor_copy(
                            kt[:], ktps[:, 0:512].rearrange("p (c x) -> p c x", c=NCH)
                        )
                    kts.append(kt)
                return kts

            kvgs = {0: emit_loads(0), 1: emit_loads(1)}
            kbfs = {0: emit_cast(kvgs[0])}
            kts_cur = emit_kt(kbfs[0])

            # ---------- main loop ----------
            for g in range(NG):
                bs = [g * GB + j for j in range(GB)]
                kvg = kvgs[g]
                kbf_cur = kbfs[g]
                if g + 2 < NG:
                    kvgs[g + 2] = emit_loads(g + 2)
                if g + 1 < NG:
                    kbfs[g + 1] = emit_cast(kvgs[g + 1])

                # scores: 4 batches packed at 32-aligned row offsets
                scps = psS.tile([128, 512], FP32, tag="scps")
                for j, b in enumerate(bs):
                    t, i = b // 8, b % 8
                    nc.tensor.matmul(
                        scps[ds(32 * j, 16), :],
                        lhsT=qT[:, t, ds(16 * i, 16)],
                        rhs=kts_cur[j][:],
                        start=True,
                        stop=True,
                        tile_position=(0, 32 * j),
                        skip_group_check=True,
                    )

                # softmax (no max subtraction: |scores| <= ~7)
                ptil = smp.tile([128, 512], FP32, tag="ptil")
                z = smallp.tile([128, 1], FP32, tag="z")
                nc.scalar.activation(
                    out=r(ptil[:]), in_=scps, func=ACTF.Exp, accum_out=z
                )

                # group g+1's transposes fill the PE bubble while the softmax
                # chain (exp -> reciprocal -> normalize) runs
                if g + 1 < NG:
                    kts_next = emit_kt(kbfs[g + 1])

                zr = smallp.tile([128, 1], FP32, tag="zr")
                nc.vector.reciprocal(zr, z)
                nc.vector.tensor_scalar_mul(r(ptil[:]), ptil, zr)

                # pT via PE transpose (f32r)
                ptps = psS.tile([128, 512], FP32, tag="scps")
                for c in range(NCH):
                    nc.tensor.transpose(
                        r(ptps[:, ts(c, 128)]), r(ptil[:, ts(c, 128)]), r(identr[:])
                    )
                pT = smp.tile([128, NCH, 128], FP32, tag="pT")
                nc.scalar.copy(r(pT[:]), ptps.rearrange("p (c x) -> p c x", c=NCH))

                # c = sum_q p (valid q columns only), scaled by SCALE
                c_t = smallp.tile([128, NCH, GB], FP32, tag="c_t")
                nc.vector.tensor_reduce(
                    out=c_t,
                    in_=pT.rearrange("p c (j w) -> p c j w", j=GB)[:, :, :, 0:16],
                    axis=AX.X,
                    op=ALU.add,
                )
                nc.vector.tensor_scalar_mul(c_t, c_t, SCALE)

                # out/kbar: [16, 512] psum halves, bf16 copies with the kbar
                # -SCALE fold done in the strided copy
                okb16 = okbp.tile([16, 1024], BF16, tag="okb16")
                okbv = okb16.rearrange("p (j h x) -> p j h x", j=4, h=2)
                for h in range(2):
                    okps = psOK.tile([16, 512], FP32, tag="okps")
                    for c in range(NCH):
                        for jj in range(2):
                            j = 2 * h + jj
                            nc.tensor.matmul(
                                okps[:, ds(256 * jj, 256)],
                                lhsT=mm(pT[:, c, ds(32 * j, 16)]),
                                rhs=mm(kvg[:, j, :, c, :]),
                                start=(c == 0 and jj == 0),
                                stop=(c == NCH - 1 and jj == 1),
                                skip_group_check=True,
                            )
                    okpsv = okps.rearrange("p (jj h x) -> p jj h x", jj=2, h=2)
                    nc.vector.tensor_scalar_mul(
                        okbv[:, ds(2 * h, 2), 0, :], okpsv[:, :, 0, :], -SCALE
                    )
                    nc.scalar.copy(okbv[:, ds(2 * h, 2), 1, :], okpsv[:, :, 1, :])

                # ---- coarse-grained chain (bf16) ----
                cgps = psCG.tile([128, 512], FP32, tag="cgps")
                for j, b in enumerate(bs):
                    t2, i2 = b // 16, b % 16
                    nc.tensor.matmul(
                        cgps[ds(32 * j, 16), 0:8],
                        lhsT=qT[:, b // 8, ds(16 * (b % 8), 16)],
                        rhs=kcgT[:, t2, ds(8 * i2, 8)],
                        start=True,
                        stop=True,
                        tile_position=(0, 32 * j),
                        skip_group_check=True,
                    )
                pcg = smallp.tile([128, 8], FP32, tag="pcg")
                zcg = smallp.tile([128, 1], FP32, tag="zcg")
                nc.scalar.activation(
                    out=r(pcg[:]), in_=cgps[:, 0:8], func=ACTF.Exp, accum_out=zcg
                )
                zcgr = smallp.tile([128, 1], FP32, tag="zcgr")
                nc.vector.reciprocal(zcgr, zcg)
                nc.vector.tensor_scalar_mul(r(pcg[:]), pcg, zcgr)
                nc.tensor.transpose(r(cgps[0:8, ds(8, 128)]), r(pcg), r(identr[:]))
                pcgT = smallp.tile([8, 128], BF16, tag="pcgT")
                nc.scalar.copy(pcgT[:], cgps[0:8, ds(8, 128)])
                nc.vector.tensor_reduce(
                    out=ccg_all[:, ds(GB * g, GB)],
                    in_=pcgT.rearrange("m (j w) -> m j w", j=GB)[:, :, 0:16],
                    axis=AX.X,
                    op=ALU.add,
                )
                # cg out/kbar through the same [16, 512] half-tile pattern
                okcg16 = okbp.tile([16, 1024], BF16, tag="okcg16")
                okcgv = okcg16.rearrange("p (j h x) -> p j h x", j=4, h=2)
                for h in range(2):
                    cgokps = psOK.tile([16, 512], FP32, tag="okps")
                    for jj in range(2):
                        j = 2 * h + jj
                        nc.tensor.matmul(
                            cgokps[:, ds(256 * jj, 256)],
                            lhsT=pcgT[:, ds(32 * j, 16)],
                            rhs=cgkv2b[:, bs[j], :],
                            start=(jj == 0),
                            stop=(jj == 1),
                            skip_group_check=True,
                        )
                    cgokv = cgokps.rearrange("p (jj h x) -> p jj h x", jj=2, h=2)
                    nc.vector.tensor_scalar_mul(
                        okcgv[:, ds(2 * h, 2), 0, :], cgokv[:, :, 0, :], -SCALE
                    )
                    nc.scalar.copy(okcgv[:, ds(2 * h, 2), 1, :], cgokv[:, :, 1, :])

                # consistency for the whole group
                dif = scrp.tile([16, 4, 128], FP32, tag="dif")
                nc.gpsimd.tensor_tensor(
                    out=dif, in0=okbv[:, :, 1, :], in1=okcgv[:, :, 1, :],
                    op=ALU.subtract,
                )
                scc = scrp.tile([16, 4, 128], FP32, tag="scc")
                nc.vector.scalar_tensor_tensor(
                    out=scc, in0=dif, scalar=1.0, in1=dif,
                    op0=ALU.mult, op1=ALU.mult,
                    accum_out=accum[0:16, ds(96 + g, 1)],
                )

                if g == 0:
                    # remaining cg casts, overlapped with group-0 compute
                    nc.scalar.copy(cgkv2b[:, 16:24], cgkv2[:, 16:24])
                    nc.vector.tensor_copy(cgkv2b[:, 24:32], cgkv2[:, 24:32])

                # ---- per-batch jacobians ----
                for j, b in enumerate(bs):
                    vc = smallp.tile([128, NCH, 128], BF16, tag="vc")
                    nc.gpsimd.tensor_tensor(
                        out=vc[:],
                        in0=kvg[:, j, 1],
                        in1=c_t[:, :, ds(j, 1)].broadcast_to([128, NCH, 128]),
                        op=ALU.mult,
                    )
                    vccg = smallp.tile([8, 128], BF16, tag="vccg")
                    nc.vector.tensor_scalar(
                        out=vccg[:],
                        in0=cgkv2b[:, b, 128:256],
                        scalar1=ccg_all[:, ds(b, 1)],
                        scalar2=SCALE,
                        op0=ALU.mult,
                        op1=ALU.mult,
                    )

                    jp = psJ.tile([128, 512], FP32, tag="jp")
                    for c in range(NCH):
                        nc.tensor.matmul(
                            jp[:, 0:128],
                            lhsT=vc[:, c, :],
                            rhs=kbf_cur[:, j, c, :],
                            start=(c == 0),
                            stop=False,
                            skip_group_check=True,
                        )
                    nc.tensor.matmul(
                        jp[:, 0:128],
                        lhsT=okb16[:, ds(256 * j + 128, 128)],
                        rhs=okb16[:, ds(256 * j, 128)],
                        start=False,
                        stop=True,
                        skip_group_check=True,
                    )
                    nc.tensor.matmul(
                        jp[:, 256:512],
                        lhsT=vccg,
                        rhs=cgkv2b[:, b, :],
                        start=True,
                        stop=False,
                        skip_group_check=True,
                    )
                    nc.tensor.matmul(
                        jp[:, 256:384],
                        lhsT=okcg16[:, ds(256 * j + 128, 128)],
                        rhs=okcg16[:, ds(256 * j, 128)],
                        start=False,
                        stop=True,
                        skip_group_check=True,
                    )

                    jd_sb = scrp.tile([128, 128], FP32, tag="jd_sb")
                    nc.vector.tensor_copy(jd_sb, jp[:, 0:128])
                    s1 = scrp.tile([128, 128], FP32, tag="s1")
                    nc.vector.scalar_tensor_tensor(
                        out=s1, in0=jd_sb, scalar=1.0, in1=jp[:, 256:384],
                        op0=ALU.mult, op1=ALU.mult,
                        accum_out=accum[:, ds(b, 1)],
                    )
                    s2 = scrp.tile([128, 128], FP32, tag="s2")
                    nc.scalar.activation(
                        out=s2, in_=jp[:, 256:384], func=ACTF.Square,
                        accum_out=accum[:, ds(64 + b, 1)],
                    )
                    s3 = scrp.tile([128, 128], FP32, tag="s3")
                    nc.scalar.activation(
                        out=s3, in_=jd_sb, func=ACTF.Square,
                        accum_out=accum[:, ds(32 + b, 1)],
                    )

                if g + 1 < NG:
                    kts_cur = kts_next

            # ---------- final reduction ----------
            rps = psJ.tile([1, 128], FP32, tag="jp")
            nc.tensor.matmul(
                rps, lhsT=ones1, rhs=accum, start=True, stop=True,
                skip_group_check=True,
            )
            row = constp.tile([1, 128], FP32)
            nc.scalar.copy(row, rps)
            f1 = constp.tile([1, 32], FP32)
            nc.vector.tensor_tensor(
                out=f1, in0=row[:, 32:64], in1=row[:, 64:96], op=ALU.mult
            )
            nc.scalar.activation(out=f1, in_=f1, func=ACTF.Sqrt)
            nc.vector.tensor_scalar_add(f1, f1, EPS)
            f2 = constp.tile([1, 32], FP32)
            nc.vector.reciprocal(f2, f1)
            nc.vector.tensor_tensor(
                out=f2, in0=row[:, 0:32], in1=f2, op=ALU.mult
            )
            csum = constp.tile([1, 1], FP32)
            nc.vector.tensor_reduce(out=csum, in_=f2, axis=AX.X, op=ALU.add)
            msum = constp.tile([1, 1], FP32)
            nc.vector.tensor_reduce(
                out=msum, in_=row[:, 96:128], axis=AX.X, op=ALU.add
            )
            part = constp.tile([1, 2], FP32)
            nc.vector.tensor_scalar(
                out=part[:, 0:1], in0=csum, scalar1=-1.0, scalar2=float(B),
                op0=ALU.mult, op1=ALU.add,
            )
            nc.vector.tensor_scalar_mul(part[:, 1:2], msum, 1.0 / QD)
            nc.sync.dma_start(out=out_d, in_=part)

    return nc


_NC_CACHE = {}


def _get_nc():
    if "nc" not in _NC_CACHE:
        _install_drain_fix()
        nc = build_nc()
        _split_waits(nc)
        _NC_CACHE["nc"] = nc
    return _NC_CACHE["nc"]


def _split_waits(nc):
    """This walrus accepts only one sync-wait per instruction; move extras
    onto same-engine NoOps inserted just before."""
    from concourse import mybir
    from bass_rust import SyncInfo

    for f in nc.m.functions:
        for blk in f.blocks:
            insts = list(blk.instructions)
            out = []
            for inst in insts:
                si = inst.sync_info
                waits = list(si.on_wait) if si is not None else []
                if len(waits) > 1:
                    for wi, w in enumerate(waits[:-1]):
                        nop = mybir.InstNoOp(name=f"{inst.name}-wsplit{wi}")
                        nop.engine = inst.engine
                        nop.sync_info = SyncInfo(on_wait=[w], on_update=[])
                        out.append(nop)
                    inst.sync_info = SyncInfo(
                        on_wait=[waits[-1]], on_update=list(si.on_update)
                    )
                out.append(inst)
            blk.instructions = out
    return nc


N_CORES = 8


def kernel(queries, keys, values, k_cg, v_cg):
    from concourse.bass_utils import run_bass_kernel_spmd

    queries = np.ascontiguousarray(np.asarray(queries, dtype=np.float32))
    keys = np.ascontiguousarray(np.asarray(keys, dtype=np.float32))
    values = np.ascontiguousarray(np.asarray(values, dtype=np.float32))
    k_cg = np.ascontiguousarray(np.asarray(k_cg, dtype=np.float32))
    v_cg = np.ascontiguousarray(np.asarray(v_cg, dtype=np.float32))

    nb = queries.shape[0]
    sh = nb // N_CORES
    in_maps = [
        {
            "queries": queries[i * sh : (i + 1) * sh],
            "keys": keys[i * sh : (i + 1) * sh],
            "values": values[i * sh : (i + 1) * sh],
            "k_cg": k_cg[i * sh : (i + 1) * sh],
            "v_cg": v_cg[i * sh : (i + 1) * sh],
        }
        for i in range(N_CORES)
    ]
    nc = _get_nc()
    res = run_bass_kernel_spmd(nc, in_maps, core_ids=list(range(N_CORES)))
    total = 0.0
    for i in range(N_CORES):
        part = res.results[i]["out"]
        total += float(part[0, 0]) + float(part[0, 1])
    return np.float32(total / nb)
